# revision 1
# baseline (speedup 1.0000x reference)
"""Trainium2 Bass kernel for nn_AttentionBlock (B=4, H=W=64, C=512).

Strategy (8 cores, no collectives):
  - 2 cores per batch image; each core handles 2048 of the 4096 queries.
  - Key/token order is permuted per core so that each core's OWN query rows
    are tokens 0..2047 of its private x copy (softmax is invariant to key
    permutation as long as K and V use the same order).
  - On-device per core: LayerNorm (stats via bn_stats), transpose hn to
    channel-major hfT via PE transposes, Q^T/K^T (channel-major) + V (token
    major) projections in bf16, then attention:
        S^T[k,q] = K^T.T @ Q^T   (PSUM fp32, scale prefolded into Wq)
        P^T = exp(S^T)           (ACT, no max subtraction: |S| < ~6)
        O[q,c]  += P^T.T @ V     (PSUM accumulate over key chunks)
        sums[q] += P^T.T @ ones
        O /= sums ; Y = (O)^T-proj via Wp ; out = Y + x + const-biases
  - LN gamma/beta are folded into the QKV weights/biases on the host;
    bv/bp biases are folded into the residual input xr on the host.
"""

import os
import sys

import numpy as np
import ml_dtypes

try:
    import concourse.bass as bass
except ImportError:  # pragma: no cover - fresh-dir fallback
    for _p in ("/opt/trn_rl_repo", "/root/.axon_site/_ro/trn_rl_repo"):
        if os.path.isdir(_p) and _p not in sys.path:
            sys.path.insert(0, _p)
    import concourse.bass as bass

import concourse.bacc as bacc
import concourse.tile as tile
from concourse import mybir
from concourse.bass_utils import run_bass_kernel_spmd

F32 = mybir.dt.float32
BF16 = mybir.dt.bfloat16
AF = mybir.ActivationFunctionType
ALU = mybir.AluOpType

B, Hh, Ww, C = 4, 64, 64, 512
N_TOK = Hh * Ww          # 4096 tokens per image
NCORES = 8
NQ = N_TOK * B // NCORES  # 2048 queries per core
LN_EPS = 1e-3
CI = C // 128            # 4 channel chunks

LAST_EXEC_NS = None
LAST_RESULT = None



def build_program(n_tok=N_TOK, nq=NQ):
    """Build the per-core Bass program (identical across cores)."""
    assert n_tok % 512 == 0 and nq % 512 == 0
    nt_tiles = n_tok // 512   # n-tiles for K/V over all tokens
    qt_tiles = nq // 512      # q-tiles for this core's queries
    kc_n = n_tok // 128       # key chunks

    nc = bacc.Bacc()
    if os.environ.get("BASS_CACHE_BUST"):
        nc.dram_tensor(f"cachebust_{os.environ['BASS_CACHE_BUST']}", [1, 1], F32)
    x_d = nc.dram_tensor("x", [n_tok, C], F32, kind="ExternalInput")
    xr_d = nc.dram_tensor("xr", [nq, C], F32, kind="ExternalInput")
    wq_d = nc.dram_tensor("wq", [C, C], BF16, kind="ExternalInput")
    wk_d = nc.dram_tensor("wk", [C, C], BF16, kind="ExternalInput")
    wv_d = nc.dram_tensor("wv", [C, C], BF16, kind="ExternalInput")
    wp_d = nc.dram_tensor("wp", [C, C], BF16, kind="ExternalInput")
    bq_d = nc.dram_tensor("bq", [128, CI], F32, kind="ExternalInput")
    bk_d = nc.dram_tensor("bk", [128, CI], F32, kind="ExternalInput")
    id_d = nc.dram_tensor("ident", [128, 128], BF16, kind="ExternalInput")
    on_d = nc.dram_tensor("ones", [128, 1], BF16, kind="ExternalInput")
    y_d = nc.dram_tensor("y", [nq, C], F32, kind="ExternalOutput")

    # token index mapping: tok = tile*512 + k*128 + p  (p = partition)
    x_re = x_d[:].rearrange("(t k p) c -> p t k c", p=128, k=4)
    xr_re = xr_d[:].rearrange("(t k p) c -> p t k c", p=128, k=4)
    y_re = y_d[:].rearrange("(t k p) c -> p t k c", p=128, k=4)

    from contextlib import ExitStack

    with ExitStack() as ctx:
        tc = ctx.enter_context(tile.TileContext(nc))
        consts = ctx.enter_context(tc.tile_pool(name="consts", bufs=1))
        big = ctx.enter_context(tc.tile_pool(name="big", bufs=1))
        work = ctx.enter_context(tc.tile_pool(name="work", bufs=3))
        stat = ctx.enter_context(tc.tile_pool(name="stat", bufs=4))
        ptp = ctx.enter_context(tc.tile_pool(name="ptp", bufs=6))
        epi = ctx.enter_context(tc.tile_pool(name="epi", bufs=3))

        # ---- first x tile + transpose identity first: they gate the
        # ---- pipeline head; weights are only needed a few µs later.
        # ---- chunk-granular DMAs let LN start after the first 256KB.
        x_t0 = work.tile([128, 4, C], F32, tag="x")
        for k in range(4):
            nc.gpsimd.dma_start(out=x_t0[:, k, :], in_=x_re[:, 0, k, :])
        ident = consts.tile([128, 128], BF16)
        nc.gpsimd.dma_start(out=ident, in_=id_d[:])

        # ---- constants ----
        wq_sb = consts.tile([128, CI, C], BF16)
        nc.gpsimd.dma_start(out=wq_sb, in_=wq_d[:].rearrange("(ci p) co -> p ci co", p=128))
        wk_sb = consts.tile([128, CI, C], BF16)
        nc.gpsimd.dma_start(out=wk_sb, in_=wk_d[:].rearrange("(ci p) co -> p ci co", p=128))
        wv_sb = consts.tile([128, CI, C], BF16)
        nc.gpsimd.dma_start(out=wv_sb, in_=wv_d[:].rearrange("(ci p) co -> p ci co", p=128))
        wp_sb = consts.tile([128, CI, C], BF16)
        nc.gpsimd.dma_start(out=wp_sb, in_=wp_d[:].rearrange("(ci p) co -> p ci co", p=128))
        bq_sb = consts.tile([128, CI], F32)
        nc.gpsimd.dma_start(out=bq_sb, in_=bq_d[:])
        bk_sb = consts.tile([128, CI], F32)
        nc.gpsimd.dma_start(out=bk_sb, in_=bk_d[:])
        ones = consts.tile([128, 1], BF16)
        nc.gpsimd.dma_start(out=ones, in_=on_d[:])
        eps_sb = consts.tile([128, 1], F32)
        nc.vector.memset(eps_sb, LN_EPS)

        # ---- persistent activations ----
        hfT = big.tile([128, CI, n_tok], BF16)   # normalized x, channel-major
        kT = big.tile([128, CI, n_tok], BF16)    # K^T, channel-major
        vN = big.tile([128, kc_n, C], BF16)      # V, token-major chunks
        qT = big.tile([128, CI, nq], BF16)       # Q^T, channel-major

        # ================= Stage A+B: LN, transpose, projections ==========
        with tc.tile_pool(name="psAB", bufs=4, space="PSUM") as psAB:
            for t in range(nt_tiles):
                if t == 0:
                    x_t = x_t0
                else:
                    x_t = work.tile([128, 4, C], F32, tag="x")
                    nc.gpsimd.dma_start(out=x_t, in_=x_re[:, t, :, :])
                # batched LN stats: one Ln + one Exp per 4 chunks
                mv4 = stat.tile([128, 4, 2], F32, tag="mv")
                for k in range(4):
                    stats = stat.tile([128, 6], F32, tag="bnst")
                    nc.vector.bn_stats(out=stats, in_=x_t[:, k, :])
                    nc.vector.bn_aggr(out=mv4[:, k, :], in_=stats)
                # rstd = exp(-0.5 * ln(var + eps))   (keeps ACT on one
                # table set: Ln/Exp/Identity/Copy all coexist)
                lnv4 = stat.tile([128, 4, 1], F32, tag="lnv")
                nc.scalar.activation(out=lnv4, in_=mv4[:, :, 1:2], func=AF.Ln,
                                     bias=eps_sb)
                rstd4 = stat.tile([128, 4, 1], F32, tag="rstd")
                nc.scalar.activation(out=rstd4, in_=lnv4, func=AF.Exp,
                                     scale=-0.5)
                for k in range(4):
                    chunk = t * 4 + k
                    hn = work.tile([128, C], BF16, tag="hn", bufs=4)
                    nc.vector.tensor_scalar(out=hn, in0=x_t[:, k, :],
                                            scalar1=mv4[:, k, 0:1],
                                            scalar2=rstd4[:, k, :],
                                            op0=ALU.subtract, op1=ALU.mult)
                    tr_ps = psAB.tile([128, CI, 128], BF16, tag="ps")
                    for j in range(CI):
                        nc.tensor.transpose(tr_ps[:, j, :],
                                            hn[:, j * 128:(j + 1) * 128], ident)
                    nc.scalar.copy(
                        out=hfT[:, :, chunk * 128:(chunk + 1) * 128], in_=tr_ps)

                # V rows for this tile's 4 chunks
                for k in range(4):
                    chunk = t * 4 + k
                    v_ps = psAB.tile([128, C], F32, tag="ps")
                    for ci in range(CI):
                        nc.tensor.matmul(
                            v_ps,
                            lhsT=hfT[:, ci, chunk * 128:(chunk + 1) * 128],
                            rhs=wv_sb[:, ci, :],
                            start=(ci == 0), stop=(ci == CI - 1))
                    if k % 2 == 0:
                        nc.vector.tensor_copy(out=vN[:, chunk, :], in_=v_ps)
                    else:
                        nc.scalar.copy(out=vN[:, chunk, :], in_=v_ps)

                # K^T columns for this tile
                for j in range(CI):
                    k_ps = psAB.tile([128, 512], F32, tag="ps")
                    for ci in range(CI):
                        nc.tensor.matmul(
                            k_ps,
                            lhsT=wk_sb[:, ci, j * 128:(j + 1) * 128],
                            rhs=hfT[:, ci, t * 512:(t + 1) * 512],
                            start=(ci == 0), stop=(ci == CI - 1))
                    if j % 2 == 0:
                        nc.scalar.activation(out=kT[:, j, t * 512:(t + 1) * 512],
                                             in_=k_ps, func=AF.Identity,
                                             bias=bk_sb[:, j:j + 1])
                    else:
                        nc.vector.tensor_scalar_add(
                            out=kT[:, j, t * 512:(t + 1) * 512],
                            in0=k_ps, scalar1=bk_sb[:, j:j + 1])

                # Q^T columns (only for this core's query range)
                if t < qt_tiles:
                    for j in range(CI):
                        q_ps = psAB.tile([128, 512], F32, tag="ps")
                        for ci in range(CI):
                            nc.tensor.matmul(
                                q_ps,
                                lhsT=wq_sb[:, ci, j * 128:(j + 1) * 128],
                                rhs=hfT[:, ci, t * 512:(t + 1) * 512],
                                start=(ci == 0), stop=(ci == CI - 1))
                        if j % 2 == 0:
                            nc.scalar.activation(
                                out=qT[:, j, t * 512:(t + 1) * 512],
                                in_=q_ps, func=AF.Identity,
                                bias=bq_sb[:, j:j + 1])
                        else:
                            nc.vector.tensor_scalar_add(
                                out=qT[:, j, t * 512:(t + 1) * 512],
                                in0=q_ps, scalar1=bq_sb[:, j:j + 1])

        # ================= Stage C: attention ============================
        with tc.tile_pool(name="psO", bufs=1, space="PSUM") as psO, \
                tc.tile_pool(name="psS", bufs=3, space="PSUM") as psS, \
                tc.tile_pool(name="psSum", bufs=1, space="PSUM") as psSum:
            for qt in range(qt_tiles):
                o_ps = psO.tile([128, 4, C], F32, tag="o")
                s_sum = psSum.tile([128, 4], F32, tag="sum")
                xr_t = work.tile([128, 4, C], F32, tag="xr")
                nc.gpsimd.dma_start(out=xr_t, in_=xr_re[:, qt, :, :])
                # software-pipelined: scores+exp for kc+1 are issued BEFORE
                # the PV matmuls of kc, so exp latency never gates PV.
                def st_exp(kc):
                    s_ps = psS.tile([128, 512], F32, tag="st",
                                    name=f"s_ps_{qt}_{kc}")
                    for ci in range(CI):
                        nc.tensor.matmul(
                            s_ps,
                            lhsT=kT[:, ci, kc * 128:(kc + 1) * 128],
                            rhs=qT[:, ci, qt * 512:(qt + 1) * 512],
                            start=(ci == 0), stop=(ci == CI - 1))
                    ptx = ptp.tile([128, 512], BF16, tag="pt",
                                   name=f"pt_{qt}_{kc}")
                    nc.scalar.activation(out=ptx, in_=s_ps, func=AF.Exp)
                    return ptx

                pts = {0: st_exp(0)}
                for kc in range(kc_n):
                    if kc + 1 < kc_n:
                        pts[kc + 1] = st_exp(kc + 1)
                    pt = pts.pop(kc)
                    for qc in range(4):
                        nc.tensor.matmul(
                            o_ps[:, qc, :],
                            lhsT=pt[:, qc * 128:(qc + 1) * 128],
                            rhs=vN[:, kc, :],
                            start=(kc == 0), stop=(kc == kc_n - 1))
                        # single accumulation group for the whole bank: the
                        # first matmul's start marks the 2KB zero-region
                        # pending-zero, so each column's first write lands as
                        # an overwrite and later writes accumulate.
                        nc.tensor.matmul(
                            s_sum[:, qc:qc + 1],
                            lhsT=pt[:, qc * 128:(qc + 1) * 128],
                            rhs=ones,
                            start=(kc == 0 and qc == 0),
                            stop=(kc == kc_n - 1 and qc == 3))
                recip = stat.tile([128, 4], F32, tag="recip")
                nc.vector.reciprocal(out=recip, in_=s_sum)
                for qc in range(4):
                    o_sb = epi.tile([128, C], BF16, tag="osb")
                    nc.vector.tensor_scalar_mul(out=o_sb, in0=o_ps[:, qc, :],
                                                scalar1=recip[:, qc:qc + 1])
                    ot_ps = psS.tile([128, CI, 128], BF16, tag="st")
                    for j in range(CI):
                        nc.tensor.transpose(ot_ps[:, j, :],
                                            o_sb[:, j * 128:(j + 1) * 128],
                                            ident)
                    ot_sb = epi.tile([128, CI, 128], BF16, tag="otsb")
                    nc.vector.tensor_copy(out=ot_sb, in_=ot_ps)
                    y_ps = psS.tile([128, C], F32, tag="st")
                    for ci in range(CI):
                        nc.tensor.matmul(y_ps, lhsT=ot_sb[:, ci, :],
                                         rhs=wp_sb[:, ci, :],
                                         start=(ci == 0), stop=(ci == CI - 1))
                    y_sb = epi.tile([128, C], F32, tag="ysb")
                    nc.vector.tensor_add(out=y_sb, in0=y_ps, in1=xr_t[:, qc, :])
                    nc.gpsimd.dma_start(out=y_re[:, qt, qc, :], in_=y_sb)

    nc.compile()
    return nc


def prep_host_inputs(x, ln_g, ln_b, Wq, bq, Wk, bk, Wv, bv, Wp, bp,
                     n_tok=N_TOK, nq=NQ, ncores=NCORES, nbatch=B):
    """Fold LN affine + scale + linear biases on the host; build per-core maps."""
    f32 = np.float32
    bf16 = ml_dtypes.bfloat16
    x = np.asarray(x, f32)
    g = np.asarray(ln_g, f32)
    b = np.asarray(ln_b, f32)
    Wq = np.asarray(Wq, f32); Wk = np.asarray(Wk, f32)
    Wv = np.asarray(Wv, f32); Wp = np.asarray(Wp, f32)
    bq = np.asarray(bq, f32); bk = np.asarray(bk, f32)
    bv = np.asarray(bv, f32); bp = np.asarray(bp, f32)

    s = 1.0 / np.sqrt(np.float32(C))
    wq_e = (g[:, None] * Wq) * s
    bq_e = (b @ Wq + bq) * s
    wk_e = g[:, None] * Wk
    bk_e = b @ Wk + bk
    wv_e = g[:, None] * Wv
    bv_e = b @ Wv + bv
    resid_const = bv_e @ Wp + bp    # [C]

    ci = C // 128
    bq_pp = np.ascontiguousarray(bq_e.reshape(ci, 128).T).astype(f32)
    bk_pp = np.ascontiguousarray(bk_e.reshape(ci, 128).T).astype(f32)
    ident = np.eye(128, dtype=bf16)
    onesv = np.ones((128, 1), dtype=bf16)

    shared = dict(
        wq=wq_e.astype(bf16), wk=wk_e.astype(bf16),
        wv=wv_e.astype(bf16), wp=Wp.astype(bf16),
        bq=bq_pp, bk=bk_pp, ident=ident, ones=onesv,
    )

    xf = x.reshape(-1, C)  # flattened tokens, nbatch * n_tok rows
    halves = ncores // nbatch
    in_maps = []
    for core in range(ncores):
        bidx, half = divmod(core, halves)
        xb = xf[bidx * n_tok:(bidx + 1) * n_tok]
        if half:
            xp = np.ascontiguousarray(
                np.concatenate([xb[half * nq:], xb[:half * nq]], axis=0))
        else:
            xp = xb
        xr = (xp[:nq] + resid_const).astype(f32)
        m = dict(shared)
        m["x"] = np.ascontiguousarray(xp)
        m["xr"] = np.ascontiguousarray(xr)
        in_maps.append(m)
    return in_maps


_PROG = None


def _get_prog():
    global _PROG
    if _PROG is None:
        _PROG = build_program()
    return _PROG


def kernel(x, ln_g, ln_b, Wq, bq, Wk, bk, Wv, bv, Wp, bp, _trace=False,
           _tmpdir=None):
    global LAST_EXEC_NS, LAST_RESULT
    nc = _get_prog()
    in_maps = prep_host_inputs(x, ln_g, ln_b, Wq, bq, Wk, bk, Wv, bv, Wp, bp)
    res = run_bass_kernel_spmd(nc, in_maps, list(range(NCORES)), trace=_trace,
                               tmpdir=_tmpdir)
    LAST_EXEC_NS = res.exec_time_ns
    LAST_RESULT = res
    y = np.empty((B, N_TOK, C), np.float32)
    halves = NCORES // B
    for core in range(NCORES):
        bidx, half = divmod(core, halves)
        y[bidx, half * NQ:(half + 1) * NQ] = res.results[core]["y"]
    return y.reshape(B, Hh, Ww, C)



# revision 4
# speedup vs baseline: 1.4299x; 1.4299x over previous
"""Trainium2 Bass kernel for nn_AttentionBlock (B=4, H=W=64, C=512).

Strategy (8 cores, no collectives):
  - 2 cores per batch image; each core handles 2048 of the 4096 queries.
  - Key/token order is permuted per core so that each core's OWN query rows
    are tokens 0..2047 of its private x copy (softmax is invariant to key
    permutation as long as K and V use the same order).
  - All GEMMs run in fp8e4 with MatmulPerfMode.DoubleRow (2x bf16 rate):
    lhsT/rhs carry [128, 2, *] channel- or key-chunk pairs so each matmul
    contracts 256 elements.
  - On-device per core: LayerNorm (stats via bn_stats), transpose hn to
    channel-major hfT (bf16 PE transpose, fp8 cast on the PSUM->SBUF copy),
    Q^T/K^T (channel-major) + V (token-major) projections in fp8, then:
        S^T[k,q] = K^T.T @ Q^T     (PSUM fp32)
        P^T = exp(S^T/sqrt(C)-1.5) (ACT, scale+shift folded into the table)
        O^T[c,q] += V.T-pair @ P^T (PSUM accumulate, no output transpose)
        sums[q]  += P^T.T @ ones
        y = (O^T/16 fp8) proj via Wp back to [q,c]; y *= 16/sums;
        out = y + x + const-biases
  - LN gamma/beta are folded into the QKV weights/biases on the host;
    bv/bp biases are folded into the residual input xr on the host; the
    softmax 1/sqrt(C) scale is applied by the ACT exp instruction.
"""

import os
import sys

import numpy as np
import ml_dtypes

try:
    import concourse.bass as bass
except ImportError:  # pragma: no cover - fresh-dir fallback
    for _p in ("/opt/trn_rl_repo", "/root/.axon_site/_ro/trn_rl_repo"):
        if os.path.isdir(_p) and _p not in sys.path:
            sys.path.insert(0, _p)
    import concourse.bass as bass

import concourse.bacc as bacc
import concourse.tile as tile
from concourse import mybir
from concourse.bass_utils import run_bass_kernel_spmd

F32 = mybir.dt.float32
BF16 = mybir.dt.bfloat16
F8 = mybir.dt.float8e4
AF = mybir.ActivationFunctionType
ALU = mybir.AluOpType
DR = mybir.MatmulPerfMode.DoubleRow
NPF8 = ml_dtypes.float8_e4m3

B, Hh, Ww, C = 4, 64, 64, 512
N_TOK = Hh * Ww          # 4096 tokens per image
NCORES = 8
NQ = N_TOK * B // NCORES  # 2048 queries per core
LN_EPS = 1e-3
CI = C // 128             # 4 channel chunks
SSCALE = 1.0 / float(np.sqrt(np.float32(C)))  # softmax scale, applied in exp
ESHIFT = -1.5             # exp(S*scale + ESHIFT): keeps P below fp8e4 max 240
OSCALE = 1.0 / 16.0       # O^T scaled into fp8 range; undone via recip16

LAST_EXEC_NS = None
LAST_RESULT = None


def build_program(n_tok=N_TOK, nq=NQ):
    """Build the per-core Bass program (identical across cores)."""
    assert n_tok % 512 == 0 and nq % 512 == 0
    nt_tiles = n_tok // 512   # n-tiles for K/V over all tokens
    qt_tiles = nq // 512      # q-tiles for this core's queries
    kc_n = n_tok // 128       # key chunks
    kp_n = kc_n // 2          # key chunk pairs

    nc = bacc.Bacc()
    if os.environ.get("BASS_CACHE_BUST"):
        nc.dram_tensor(f"cachebust_{os.environ['BASS_CACHE_BUST']}", [1, 1], F32)
    x_d = nc.dram_tensor("x", [n_tok, C], F32, kind="ExternalInput")
    xr_d = nc.dram_tensor("xr", [nq, C], F32, kind="ExternalInput")
    wq_d = nc.dram_tensor("wq", [C, C], F8, kind="ExternalInput")
    wk_d = nc.dram_tensor("wk", [C, C], F8, kind="ExternalInput")
    wv_d = nc.dram_tensor("wv", [C, C], F8, kind="ExternalInput")
    wp_d = nc.dram_tensor("wp", [C, C], F8, kind="ExternalInput")
    bq_d = nc.dram_tensor("bq", [128, CI], F32, kind="ExternalInput")
    bk_d = nc.dram_tensor("bk", [128, CI], F32, kind="ExternalInput")
    id_d = nc.dram_tensor("ident", [128, 128], BF16, kind="ExternalInput")
    on_d = nc.dram_tensor("ones", [128, 2, 1], F8, kind="ExternalInput")
    y_d = nc.dram_tensor("y", [nq, C], F32, kind="ExternalOutput")

    # token index mapping: tok = tile*512 + k*128 + p  (p = partition)
    x_re = x_d[:].rearrange("(t k p) c -> p t k c", p=128, k=4)
    xr_re = xr_d[:].rearrange("(t k p) c -> p t k c", p=128, k=4)
    y_re = y_d[:].rearrange("(t k p) c -> p t k c", p=128, k=4)

    from contextlib import ExitStack

    with ExitStack() as ctx:
        tc = ctx.enter_context(tile.TileContext(nc))
        consts = ctx.enter_context(tc.tile_pool(name="consts", bufs=1))
        big = ctx.enter_context(tc.tile_pool(name="big", bufs=1))
        work = ctx.enter_context(tc.tile_pool(name="work", bufs=3))
        stat = ctx.enter_context(tc.tile_pool(name="stat", bufs=4))
        ptp = ctx.enter_context(tc.tile_pool(name="ptp", bufs=4))
        epi = ctx.enter_context(tc.tile_pool(name="epi", bufs=3))

        # ---- first x tile + transpose identity first: they gate the
        # ---- pipeline head; weights are only needed a few µs later.
        x_t0 = work.tile([128, 4, C], F32, tag="x")
        for k in range(4):
            nc.gpsimd.dma_start(out=x_t0[:, k, :], in_=x_re[:, 0, k, :])
        ident = consts.tile([128, 128], BF16)
        nc.gpsimd.dma_start(out=ident, in_=id_d[:])

        # ---- constants ----
        wq_sb = consts.tile([128, CI, C], F8)
        nc.gpsimd.dma_start(out=wq_sb, in_=wq_d[:].rearrange("(ci p) co -> p ci co", p=128))
        wk_sb = consts.tile([128, CI, C], F8)
        nc.gpsimd.dma_start(out=wk_sb, in_=wk_d[:].rearrange("(ci p) co -> p ci co", p=128))
        wv_sb = consts.tile([128, CI, C], F8)
        nc.gpsimd.dma_start(out=wv_sb, in_=wv_d[:].rearrange("(ci p) co -> p ci co", p=128))
        wp_sb = consts.tile([128, CI, C], F8)
        nc.gpsimd.dma_start(out=wp_sb, in_=wp_d[:].rearrange("(ci p) co -> p ci co", p=128))
        bq_sb = consts.tile([128, CI], F32)
        nc.gpsimd.dma_start(out=bq_sb, in_=bq_d[:])
        bk_sb = consts.tile([128, CI], F32)
        nc.gpsimd.dma_start(out=bk_sb, in_=bk_d[:])
        ones8 = consts.tile([128, 2, 1], F8)
        nc.gpsimd.dma_start(out=ones8, in_=on_d[:])
        eps_sb = consts.tile([128, 1], F32)
        nc.vector.memset(eps_sb, LN_EPS)
        shf_sb = consts.tile([128, 1], F32)
        nc.vector.memset(shf_sb, ESHIFT)

        # ---- persistent activations (all fp8, channel pairs sliceable) ----
        hfT = big.tile([128, CI, n_tok], F8)     # normalized x, channel-major
        kT = big.tile([128, CI, n_tok], F8)      # K^T, channel-major
        vN = big.tile([128, kc_n, C], F8)        # V, token-major chunks
        qT = big.tile([128, CI, nq], F8)         # Q^T, channel-major

        # ================= Stage A+B: LN, transpose, projections ==========
        with tc.tile_pool(name="psAB", bufs=4, space="PSUM") as psAB:
            for t in range(nt_tiles):
                if t == 0:
                    x_t = x_t0
                else:
                    x_t = work.tile([128, 4, C], F32, tag="x")
                    nc.gpsimd.dma_start(out=x_t, in_=x_re[:, t, :, :])
                # batched LN stats: one Ln + one Exp per 4 chunks
                mv4 = stat.tile([128, 4, 2], F32, tag="mv")
                for k in range(4):
                    stats = stat.tile([128, 6], F32, tag="bnst")
                    nc.vector.bn_stats(out=stats, in_=x_t[:, k, :])
                    nc.vector.bn_aggr(out=mv4[:, k, :], in_=stats)
                # rstd = exp(-0.5 * ln(var + eps))   (keeps ACT on one
                # table set: Ln/Exp/Identity/Copy all coexist)
                lnv4 = stat.tile([128, 4, 1], F32, tag="lnv")
                nc.scalar.activation(out=lnv4, in_=mv4[:, :, 1:2], func=AF.Ln,
                                     bias=eps_sb)
                rstd4 = stat.tile([128, 4, 1], F32, tag="rstd")
                nc.scalar.activation(out=rstd4, in_=lnv4, func=AF.Exp,
                                     scale=-0.5)
                for k in range(4):
                    chunk = t * 4 + k
                    hn = work.tile([128, C], BF16, tag="hn", bufs=4)
                    nc.vector.tensor_scalar(out=hn, in0=x_t[:, k, :],
                                            scalar1=mv4[:, k, 0:1],
                                            scalar2=rstd4[:, k, :],
                                            op0=ALU.subtract, op1=ALU.mult)
                    tr_ps = psAB.tile([128, CI, 128], BF16, tag="ps")
                    for j in range(CI):
                        nc.tensor.transpose(tr_ps[:, j, :],
                                            hn[:, j * 128:(j + 1) * 128], ident)
                    # fp8 cast on the PSUM->SBUF copy
                    if k % 2 == 0:
                        nc.scalar.copy(
                            out=hfT[:, :, chunk * 128:(chunk + 1) * 128],
                            in_=tr_ps)
                    else:
                        nc.vector.tensor_copy(
                            out=hfT[:, :, chunk * 128:(chunk + 1) * 128],
                            in_=tr_ps)

                # V rows for this tile's 4 chunks (DoubleRow over ci pairs)
                for k in range(4):
                    chunk = t * 4 + k
                    v_ps = psAB.tile([128, C], F32, tag="ps")
                    for ip in range(CI // 2):
                        nc.tensor.matmul(
                            v_ps,
                            lhsT=hfT[:, 2 * ip:2 * ip + 2,
                                     chunk * 128:(chunk + 1) * 128],
                            rhs=wv_sb[:, 2 * ip:2 * ip + 2, :],
                            perf_mode=DR,
                            start=(ip == 0), stop=(ip == CI // 2 - 1))
                    if k % 2 == 0:
                        nc.vector.tensor_copy(out=vN[:, chunk, :], in_=v_ps)
                    else:
                        nc.scalar.copy(out=vN[:, chunk, :], in_=v_ps)

                # K^T columns for this tile
                for j in range(CI):
                    k_ps = psAB.tile([128, 512], F32, tag="ps")
                    for ip in range(CI // 2):
                        nc.tensor.matmul(
                            k_ps,
                            lhsT=wk_sb[:, 2 * ip:2 * ip + 2,
                                       j * 128:(j + 1) * 128],
                            rhs=hfT[:, 2 * ip:2 * ip + 2,
                                    t * 512:(t + 1) * 512],
                            perf_mode=DR,
                            start=(ip == 0), stop=(ip == CI // 2 - 1))
                    if j % 2 == 0:
                        nc.scalar.activation(out=kT[:, j, t * 512:(t + 1) * 512],
                                             in_=k_ps, func=AF.Identity,
                                             bias=bk_sb[:, j:j + 1])
                    else:
                        nc.vector.tensor_scalar_add(
                            out=kT[:, j, t * 512:(t + 1) * 512],
                            in0=k_ps, scalar1=bk_sb[:, j:j + 1])

                # Q^T columns (only for this core's query range)
                if t < qt_tiles:
                    for j in range(CI):
                        q_ps = psAB.tile([128, 512], F32, tag="ps")
                        for ip in range(CI // 2):
                            nc.tensor.matmul(
                                q_ps,
                                lhsT=wq_sb[:, 2 * ip:2 * ip + 2,
                                           j * 128:(j + 1) * 128],
                                rhs=hfT[:, 2 * ip:2 * ip + 2,
                                        t * 512:(t + 1) * 512],
                                perf_mode=DR,
                                start=(ip == 0), stop=(ip == CI // 2 - 1))
                        if j % 2 == 0:
                            nc.scalar.activation(
                                out=qT[:, j, t * 512:(t + 1) * 512],
                                in_=q_ps, func=AF.Identity,
                                bias=bq_sb[:, j:j + 1])
                        else:
                            nc.vector.tensor_scalar_add(
                                out=qT[:, j, t * 512:(t + 1) * 512],
                                in0=q_ps, scalar1=bq_sb[:, j:j + 1])

        # ================= Stage C: attention ============================
        with tc.tile_pool(name="psO", bufs=1, space="PSUM") as psO, \
                tc.tile_pool(name="psS", bufs=3, space="PSUM") as psS, \
                tc.tile_pool(name="psSum", bufs=1, space="PSUM") as psSum:
            for qt in range(qt_tiles):
                oT_ps = psO.tile([128, 4, C], F32, tag="o")   # [c-part, cj, q]
                s_sum = psSum.tile([128, 4], F32, tag="sum")
                xr_t = work.tile([128, 4, C], F32, tag="xr")
                nc.gpsimd.dma_start(out=xr_t, in_=xr_re[:, qt, :, :])

                # software-pipelined: scores+exp for pair p+1 are issued
                # BEFORE the PV matmuls of pair p, so exp never gates PV.
                def st_exp(kc, pt2, plane):
                    s_ps = psS.tile([128, 512], F32, tag="st",
                                    name=f"s_ps_{qt}_{kc}")
                    for ip in range(CI // 2):
                        nc.tensor.matmul(
                            s_ps,
                            lhsT=kT[:, 2 * ip:2 * ip + 2,
                                    kc * 128:(kc + 1) * 128],
                            rhs=qT[:, 2 * ip:2 * ip + 2,
                                   qt * 512:(qt + 1) * 512],
                            perf_mode=DR,
                            start=(ip == 0), stop=(ip == CI // 2 - 1))
                    nc.scalar.activation(out=pt2[:, plane, :], in_=s_ps,
                                         func=AF.Exp, scale=SSCALE,
                                         bias=shf_sb)

                def make_pair(p):
                    pt2 = ptp.tile([128, 2, 512], F8, tag="pt",
                                   name=f"pt_{qt}_{p}")
                    st_exp(2 * p, pt2, 0)
                    st_exp(2 * p + 1, pt2, 1)
                    return pt2

                pts = {0: make_pair(0)}
                for p in range(kp_n):
                    if p + 1 < kp_n:
                        pts[p + 1] = make_pair(p + 1)
                    pt2 = pts.pop(p)
                    for cj in range(4):
                        nc.tensor.matmul(
                            oT_ps[:, cj, :],
                            lhsT=vN[:, 2 * p:2 * p + 2,
                                    cj * 128:(cj + 1) * 128],
                            rhs=pt2[:],
                            perf_mode=DR,
                            start=(p == 0), stop=(p == kp_n - 1))
                    for qc in range(4):
                        # single accumulation group for the whole bank: the
                        # first matmul's start marks the 2KB zero-region
                        # pending-zero, so each column's first write lands as
                        # an overwrite and later writes accumulate.
                        nc.tensor.matmul(
                            s_sum[:, qc:qc + 1],
                            lhsT=pt2[:, :, qc * 128:(qc + 1) * 128],
                            rhs=ones8,
                            perf_mode=DR,
                            start=(p == 0 and qc == 0),
                            stop=(p == kp_n - 1 and qc == 3))

                # ---- epilogue: scale O^T into fp8, project back, add x ----
                ssc = stat.tile([128, 4], F32, tag="ssc")
                nc.scalar.mul(ssc, s_sum, OSCALE)     # s_sum/16
                recip16 = stat.tile([128, 4], F32, tag="recip")
                nc.vector.reciprocal(out=recip16, in_=ssc)   # 16/s_sum
                oT8 = epi.tile([128, 4, C], F8, tag="ot8")
                for cj in range(4):  # one ACT per PSUM bank
                    nc.scalar.mul(oT8[:, cj, :], oT_ps[:, cj, :], OSCALE)
                for qc in range(4):
                    y_ps = psS.tile([128, C], F32, tag="st")
                    for ip in range(CI // 2):
                        nc.tensor.matmul(
                            y_ps,
                            lhsT=oT8[:, 2 * ip:2 * ip + 2,
                                     qc * 128:(qc + 1) * 128],
                            rhs=wp_sb[:, 2 * ip:2 * ip + 2, :],
                            perf_mode=DR,
                            start=(ip == 0), stop=(ip == CI // 2 - 1))
                    y_sc = epi.tile([128, C], F32, tag="ysc")
                    nc.scalar.activation(out=y_sc, in_=y_ps, func=AF.Copy,
                                         scale=recip16[:, qc:qc + 1])
                    y_sb = epi.tile([128, C], F32, tag="ysb")
                    nc.vector.tensor_add(out=y_sb, in0=y_sc, in1=xr_t[:, qc, :])
                    nc.gpsimd.dma_start(out=y_re[:, qt, qc, :], in_=y_sb)

    nc.compile()
    return nc


def prep_host_inputs(x, ln_g, ln_b, Wq, bq, Wk, bk, Wv, bv, Wp, bp,
                     n_tok=N_TOK, nq=NQ, ncores=NCORES, nbatch=B):
    """Fold LN affine + linear biases on the host; build per-core maps."""
    f32 = np.float32
    x = np.asarray(x, f32)
    g = np.asarray(ln_g, f32)
    b = np.asarray(ln_b, f32)
    Wq = np.asarray(Wq, f32); Wk = np.asarray(Wk, f32)
    Wv = np.asarray(Wv, f32); Wp = np.asarray(Wp, f32)
    bq = np.asarray(bq, f32); bk = np.asarray(bk, f32)
    bv = np.asarray(bv, f32); bp = np.asarray(bp, f32)

    wq_e = g[:, None] * Wq
    bq_e = b @ Wq + bq
    wk_e = g[:, None] * Wk
    bk_e = b @ Wk + bk
    wv_e = g[:, None] * Wv
    bv_e = b @ Wv + bv
    resid_const = bv_e @ Wp + bp    # [C]

    ci = C // 128
    bq_pp = np.ascontiguousarray(bq_e.reshape(ci, 128).T).astype(f32)
    bk_pp = np.ascontiguousarray(bk_e.reshape(ci, 128).T).astype(f32)
    ident = np.eye(128, dtype=ml_dtypes.bfloat16)
    onesv = np.ones((128, 2, 1), dtype=NPF8)

    shared = dict(
        wq=wq_e.astype(NPF8), wk=wk_e.astype(NPF8),
        wv=wv_e.astype(NPF8), wp=Wp.astype(NPF8),
        bq=bq_pp, bk=bk_pp, ident=ident, ones=onesv,
    )

    xf = x.reshape(-1, C)  # flattened tokens, nbatch * n_tok rows
    halves = ncores // nbatch
    in_maps = []
    for core in range(ncores):
        bidx, half = divmod(core, halves)
        xb = xf[bidx * n_tok:(bidx + 1) * n_tok]
        if half:
            xp = np.ascontiguousarray(
                np.concatenate([xb[half * nq:], xb[:half * nq]], axis=0))
        else:
            xp = xb
        xr = (xp[:nq] + resid_const).astype(f32)
        m = dict(shared)
        m["x"] = np.ascontiguousarray(xp)
        m["xr"] = np.ascontiguousarray(xr)
        in_maps.append(m)
    return in_maps


_PROG = None


def _get_prog():
    global _PROG
    if _PROG is None:
        _PROG = build_program()
    return _PROG


def kernel(x, ln_g, ln_b, Wq, bq, Wk, bk, Wv, bv, Wp, bp, _trace=False,
           _tmpdir=None):
    global LAST_EXEC_NS, LAST_RESULT
    nc = _get_prog()
    in_maps = prep_host_inputs(x, ln_g, ln_b, Wq, bq, Wk, bk, Wv, bv, Wp, bp)
    res = run_bass_kernel_spmd(nc, in_maps, list(range(NCORES)), trace=_trace,
                               tmpdir=_tmpdir)
    LAST_EXEC_NS = res.exec_time_ns
    LAST_RESULT = res
    y = np.empty((B, N_TOK, C), np.float32)
    halves = NCORES // B
    for core in range(NCORES):
        bidx, half = divmod(core, halves)
        y[bidx, half * NQ:(half + 1) * NQ] = res.results[core]["y"]
    return y.reshape(B, Hh, Ww, C)
